# revision 22
# baseline (speedup 1.0000x reference)
"""Transformer block (pre-norm attn + MLP) on 8 NeuronCores, data-parallel
over batch. Full inputs in, full outputs out; each core runs one batch
element x[i] : [1024, 768] through an identical Bass/Tile kernel.

v3: attention path in fp8 (e4m3) with DoubleRow matmuls.

Host-side exact refactoring (as v2) plus fp8 quantization of the
attention weights:
  - LN gains fold into the following matmul weights: diag(g) @ W.
  - LN biases fold into: per-column bias on q/k (x16), b_proj_eff,
    b_fc1_eff.
  - w_qkv, w_proj quantized to e4m3 at scale 512 (max |w|*512 ~ 100).
  - w_proj rows re-laid-out head-aligned: block h rows 1..96 (row 0 pairs
    with the attention colsum row; zero).
  - x cast to bf16; MLP weights bf16 (fp8 MLP breaches the 2e-2 gate).

fp8 scaling scheme (S_A = 16 activations, S_W = 512 weights):
  h_fm8 = 16*ln1(x) -> q8/k8 = 16*q (psum/512 + 16*bias), v8 = 16*v.
  scores psum = 256*(q.k); exp arg = psum*SCALE/256 - 4 (the -4 keeps
  e below e4m3 max 240; softmax is shift-invariant incl. the colsum row).
  e8 = exp(s-4); PV row0 = colsum; o8 = 16*o after reciprocal bcast.
  proj psum = 16*512*(o@wp) -> residual add with 1/8192 fold (stt).

DoubleRow (contraction 256/matmul, 2x PE throughput) on qkv, v, PV (m-tile
pairs), proj (head-block pairs). Scores stay at 96-contraction (fp8 at
bf16 rate) in this version.

Device dataflow per core: as v2 (single fully-pipelined emission).
"""
import numpy as np
import ml_dtypes

import concourse.bass as bass
from concourse import bacc, mybir
from concourse.bass_utils import run_bass_kernel_spmd
from concourse.masks import make_identity
from concourse.tile import TileContext

P = 128
N = 1024          # tokens per core (batch element)
C = 768           # model dim
H = 8             # heads
DH = C // H       # 96
DFF = 4 * C       # 3072
NT = N // P       # 8 token tiles
KT = C // P       # 6 feature tiles
FFT = DFF // P    # 24 ff tiles
NH = 2            # halves of the token axis for attention
NC_ = N // NH     # 512
EPS = 1e-5
SCALE = DH ** -0.5
VW = DH           # per-head v width (plus a leading ones column)
VWP = 104         # padded per-head v stride: 8*104 % 16 == 0 (DoubleRow
                  # pair-axis step must be a 16B multiple)
S_A = 16.0        # fp8 activation scale
S_W = 512.0       # fp8 weight scale
ESHIFT = -4.0     # exp downshift so e stays < 240

F32 = mybir.dt.float32
BF16 = mybir.dt.bfloat16
F8 = mybir.dt.float8e4
MULT = mybir.AluOpType.mult
ADD = mybir.AluOpType.add
DR = mybir.MatmulPerfMode.DoubleRow

_CACHED = {}


def build(taps=()):
    nc = bacc.Bacc("TRN2", debug=False)

    x_d = nc.dram_tensor("x_bf", [N, C], BF16, kind="ExternalInput")
    wqkv_d = nc.dram_tensor("w_qkv_e", [C, 3 * C], F8, kind="ExternalInput")
    wproj_d = nc.dram_tensor("w_proj_p", [H * P, C], F8, kind="ExternalInput")
    wfc1_d = nc.dram_tensor("w_fc1_e", [C, DFF], BF16, kind="ExternalInput")
    wfc18_d = nc.dram_tensor("w_fc1_8", [2 * P, DFF], F8, kind="ExternalInput")
    wfc2_d = nc.dram_tensor("w_fc2", [DFF, C], BF16, kind="ExternalInput")
    wfc28_d = nc.dram_tensor("w_fc2_8", [4 * P, C], F8, kind="ExternalInput")
    qkb_d = nc.dram_tensor("qk_bias", [P, 2 * H], F32, kind="ExternalInput")
    bp_d = nc.dram_tensor("b_proj_e", [C], BF16, kind="ExternalInput")
    bf1_d = nc.dram_tensor("b_fc1_e", [DFF], F32, kind="ExternalInput")
    bf2_d = nc.dram_tensor("b_fc2", [C], BF16, kind="ExternalInput")
    y_d = nc.dram_tensor("y", [N, C], BF16, kind="ExternalOutput")

    tap_d = {}
    for name, shape, dt in [
        ("h_fm", [C, N], F8),
        ("q_fm", [H * P, N], F8),
        ("k_fm", [H * P, N], F8),
        ("o_fm", [H * P, N], F8),
        ("x1", [N, C], BF16),
        ("h2_fm", [C, N], BF16),
    ]:
        if name in taps:
            tap_d[name] = nc.dram_tensor(
                "tap_" + name, shape, dt, kind="ExternalOutput"
            )

    with TileContext(nc) as tc:
        # ---------------- SBUF pools, LEFT stack (bottom -> top) --------
        consts = tc.alloc_tile_pool(name="consts", bufs=1, side="left")
        xpool = tc.alloc_tile_pool(name="xpool", bufs=1, side="left")
        wprojp = tc.alloc_tile_pool(name="wprojp", bufs=1, side="left")
        h2p = tc.alloc_tile_pool(name="h2p", bufs=1, side="left")
        lnscr = tc.alloc_tile_pool(name="lnscr", bufs=2, side="left")
        hfmp = tc.alloc_tile_pool(name="hfmp", bufs=1, side="left")
        wqkvp = tc.alloc_tile_pool(name="wqkvp", bufs=1, side="left")

        # ---------------- SBUF pools, RIGHT stack -----------------------
        opool = tc.alloc_tile_pool(name="opool", bufs=1, side="right")
        rrow = tc.alloc_tile_pool(name="rrow", bufs=1, side="right")
        vpool = tc.alloc_tile_pool(name="vpool", bufs=1, side="right")
        gpool = tc.alloc_tile_pool(name="gpool", bufs=1, side="right")
        epool = tc.alloc_tile_pool(name="epool", bufs=8, side="right")
        qpool = tc.alloc_tile_pool(name="qpool", bufs=1, side="right")
        kpool = tc.alloc_tile_pool(name="kpool", bufs=1, side="right")


        # ---------------- PSUM pools ------------------------------------
        work1 = tc.alloc_tile_pool(name="work1", bufs=2, space="PSUM")
        tpps = tc.alloc_tile_pool(name="tpps", bufs=2, space="PSUM")
        sps = tc.alloc_tile_pool(name="sps", bufs=2, space="PSUM")

        # ---------------- constants ------------------------------------
        ident = consts.tile([P, P], BF16)
        make_identity(nc, ident)
        eps_t = consts.tile([P, 1], F32)
        nc.vector.memset(eps_t, EPS)
        esh_t = consts.tile([P, 1], F32)
        nc.vector.memset(esh_t, ESHIFT)
        dum = consts.tile([1, 1], F32)
        qkb = consts.tile([P, 2 * H], F32)
        bf1c = consts.tile([P, FFT], F32)
        bpb = consts.tile([P, C], BF16)
        bf2b = consts.tile([P, C], BF16)

        # ---------------- big tiles + DMAs ------------------------------
        # spread across engine queues: each queue moves ~130 GB/s, so the
        # startup loads (x then wq/wk) go wide, ordered by first consumer
        x_tok = xpool.tile([P, NT, C], BF16)
        xr = x_d.rearrange("(nt p) c -> p nt c", p=P)
        wqkv = wqkvp.tile([P, KT, 3 * C], F8)
        wr = wqkv_d.rearrange("(kt p) o -> p kt o", p=P)
        # per-tile x chunks across all 5 queues: tile nt lands ~1.3us after
        # queue start, so the LN1 stats chain starts ~4us earlier than with
        # 2-3-tile chunks.  wqkv interleaves behind x in consumption order
        # (q cols first, then k, then v).
        # Scalar (ACT) engine shares its queue between DMA triggers and
        # ACTIVATE ops, and triggers block on ring credits -- so the scalar
        # ring carries only tiny/early transfers, keeping the LN1 identity
        # chain unblocked.  Big loads ride sync+gpsimd in consumption order.
        nc.sync.dma_start(x_tok[:, 0, 0:384], xr[:, 0, 0:384])
        nc.gpsimd.dma_start(x_tok[:, 1, 0:384], xr[:, 1, 0:384])
        nc.sync.dma_start(x_tok[:, 0, 384:C], xr[:, 0, 384:C])
        nc.gpsimd.dma_start(x_tok[:, 1, 384:C], xr[:, 1, 384:C])
        nc.scalar.dma_start(qkb[:], qkb_d[:, :])
        nc.sync.dma_start(x_tok[:, 2, :], xr[:, 2, :])
        nc.gpsimd.dma_start(x_tok[:, 3, :], xr[:, 3, :])
        nc.scalar.dma_start(x_tok[:, 6, :], xr[:, 6, :])
        nc.scalar.dma_start(x_tok[:, 7, :], xr[:, 7, :])
        nc.sync.dma_start(x_tok[:, 4, :], xr[:, 4, :])
        nc.gpsimd.dma_start(x_tok[:, 5, :], xr[:, 5, :])
        nc.sync.dma_start(wqkv[:, :, 0:384], wr[:, :, 0:384])
        nc.gpsimd.dma_start(wqkv[:, :, 384:C], wr[:, :, 384:C])
        nc.sync.dma_start(wqkv[:, :, C:C + 384], wr[:, :, C:C + 384])
        nc.gpsimd.dma_start(wqkv[:, :, C + 384:2 * C], wr[:, :, C + 384:2 * C])
        nc.sync.dma_start(wqkv[:, :, 2 * C:2 * C + 384], wr[:, :, 2 * C:2 * C + 384])
        nc.gpsimd.dma_start(wqkv[:, :, 2 * C + 384:3 * C], wr[:, :, 2 * C + 384:])

        brow1 = consts.tile([1, C], BF16)
        brow2 = consts.tile([1, C], BF16)
        nc.scalar.dma_start(
            brow1[0:1, :], bass.AP(tensor=bp_d, offset=0, ap=[[0, 1], [1, C]])
        )
        nc.scalar.dma_start(
            brow2[0:1, :], bass.AP(tensor=bf2_d, offset=0, ap=[[0, 1], [1, C]])
        )
        nc.gpsimd.partition_broadcast(bpb[:, :], brow1[0:1, :])
        nc.gpsimd.partition_broadcast(bf2b[:, :], brow2[0:1, :])

        wproj = wprojp.tile([P, H, C], F8)
        nc.sync.dma_start(wproj[:], wproj_d.rearrange("(hb p) c -> p hb c", p=P))
        nc.scalar.dma_start(bf1c[:], bf1_d.rearrange("(t p) -> p t", p=P))

        # Load the exp table set immediately (PE is idle at t=0); every
        # later ACT op until the MLP (exp) uses this same set.
        nc.scalar.activation(
            out=dum[0:1, 0:1], in_=eps_t[0:1, 0:1],
            func=mybir.ActivationFunctionType.Exp, bias=0.0, scale=1.0,
        )

        h_fm = hfmp.tile([P, KT, N], F8)
        h2_fm = h2p.tile([P, 4, N], BF16)
        h2_fm8 = h2p.tile([P, 2, N], F8)
        o_fm = opool.tile([P, H, N], F8)
        q_fm = qpool.tile([P, H, N], F8)
        k_fm = kpool.tile([P, H, N], F8)
        v_ext = vpool.tile([P, NT, H, VWP], F8)
        nc.gpsimd.memset(v_ext[:, :, :, 0], 1.0)

        # ---------------- helpers ---------------------------------------
        def emit_ln_stats(nt, mv_out):
            """bn stats of x_tok[:, nt, :] -> mv_out [P, 2] (mean, var)."""
            st = lnscr.tile([P, 2, nc.vector.BN_STATS_DIM], F32, tag="st")
            for i in range(2):
                nc.vector.bn_stats(
                    out=st[:, i, :], in_=x_tok[:, nt, i * 384:(i + 1) * 384]
                )
            nc.vector.bn_aggr(out=mv_out, in_=st[:])

        def emit_newton(vars_, rstds, w, iters=3, final_scale=None):
            """rstds[:, :w] = 1/sqrt(vars_[:, :w] + EPS) on DVE, batched.
            var is ~1 here (layernorm of ~unit-variance activations over
            768 dims), so a linear seed + 3 Newton steps converge to float
            accuracy.  final_scale folds the fp8 activation scale in."""
            vp = lnscr.tile([P, 4], F32, tag="vp")
            nc.vector.tensor_scalar_add(vp[:, :w], vars_[:, :w], EPS)
            nc.vector.tensor_scalar(
                rstds[:, :w], vp[:, :w], -0.5, 1.5, MULT, ADD
            )
            for it in range(iters):
                t = lnscr.tile([P, 4], F32, tag="nt")
                nc.vector.tensor_mul(t[:, :w], rstds[:, :w], rstds[:, :w])
                nc.vector.tensor_mul(t[:, :w], t[:, :w], vp[:, :w])
                nc.vector.tensor_scalar(
                    t[:, :w], t[:, :w], -0.5, 1.5, MULT, ADD
                )
                if final_scale is not None and it == iters - 1:
                    nc.vector.tensor_scalar_mul(t[:, :w], t[:, :w], final_scale)
                nc.vector.tensor_mul(rstds[:, :w], rstds[:, :w], t[:, :w])

        def emit_badd(nt, brow):
            """x_tok[:, nt, :] += brow (after the LN that reads the
            pre-bias value, before the residual add that needs it)."""
            nc.vector.tensor_add(
                x_tok[:, nt, :], x_tok[:, nt, :], brow[:]
            )

        def emit_ln2(nt):
            """full per-tile LN2 (stats + per-tile Newton + DVE apply +
            transposes into h2_fm).  DVE apply: the ACT identity path is
            reserved for the attention-window exp stream."""
            mv = lnscr.tile([P, 2], F32, tag="mv2")
            rst = lnscr.tile([P, 1], F32, tag="rst2")
            emit_ln_stats(nt, mv[:])
            emit_newton(mv[:, 1:2], rst[:, 0:1], 1)
            nmu = lnscr.tile([P, 1], F32, tag="nmu2")
            nc.vector.tensor_scalar_mul(nmu[:], mv[:, 0:1], -1.0)
            h_t = lnscr.tile([P, C], BF16, tag="h")
            nc.vector.tensor_scalar(
                h_t[:], x_tok[:, nt, :], nmu[:], rst[:, 0:1], ADD, MULT
            )
            for kt in range(KT):
                tp = tpps.tile([P, P], BF16, tag="tp")
                nc.tensor.transpose(
                    tp[:], h_t[:, kt * P:(kt + 1) * P], ident[:]
                )
                dst = (h2_fm[:, kt, nt * P:(nt + 1) * P] if kt < 4
                       else h2_fm8[:, kt - 4, nt * P:(nt + 1) * P])
                nc.vector.tensor_copy(dst, tp[:])
            emit_badd(nt, bf2b)

        def emit_qk1(h, which, nh):
            """q or k for one head/half: DoubleRow fp8, psum/512 + 16*bias
            -> fp8 stage [96, 512], then two SBUF->SBUF shift DMAs into the
            packed pair layout (partition moves need DMA)."""
            dst = q_fm if which == 0 else k_fm
            col0 = which * C + h * DH
            pq = work1.tile([P, NC_], F32, tag="w",
                            name=f"qk_{h}_{which}_{nh}")
            for kp in range(KT // 2):
                nc.tensor.matmul(
                    pq[:DH, :],
                    wqkv[:, 2 * kp:2 * kp + 2, col0:col0 + DH],
                    h_fm[:, 2 * kp:2 * kp + 2, nh * NC_:(nh + 1) * NC_],
                    start=(kp == 0), stop=(kp == KT // 2 - 1),
                    perf_mode=DR,
                )
            nc.vector.tensor_scalar(
                dst[:DH, h, nh * NC_:(nh + 1) * NC_],
                pq[:DH, :],
                1.0 / S_W,
                qkb[:DH, which * H + h:which * H + h + 1],
                MULT, ADD,
            )

        def emit_v(nt, half):
            pv = work1.tile([P, NC_], F32, tag="w", name=f"v_{nt}_{half}")
            c0 = 2 * C + half * 4 * DH
            for kp in range(KT // 2):
                nc.tensor.matmul(
                    pv[:, 0:4 * DH],
                    h_fm[:, 2 * kp:2 * kp + 2, nt * P:(nt + 1) * P],
                    wqkv[:, 2 * kp:2 * kp + 2, c0:c0 + 4 * DH],
                    start=(kp == 0), stop=(kp == KT // 2 - 1),
                    perf_mode=DR,
                )
            nc.vector.tensor_scalar_mul(
                v_ext[:, nt, half * 4:(half + 1) * 4, 1:VW + 1],
                pv[:, 0:4 * DH].rearrange("p (h d) -> p h d", d=DH),
                1.0 / S_W,
            )

        def emit_scores_begin(h, nh):
            return epool.tile([P, NT, NC_], F8, tag="E", name=f"e_{h}_{nh}")

        def emit_scores_fill_pair(eA, eB, i, nh, mt2):
            """scores for head pair (2i, 2i+1), m-subtiles 2mt2..2mt2+1.
            Plain fp8 matmuls (contraction 96): DoubleRow here is
            LDWEIGHTS-bound (the k m-slice weights change every matmul)."""
            nsl = slice(nh * NC_, (nh + 1) * NC_)
            for h, e_t in ((2 * i, eA), (2 * i + 1, eB)):
                ps = sps.tile([P, 2, NC_], F32, tag="S",
                              name=f"s_{h}_{nh}_{mt2}")
                for sub in range(2):
                    m0 = (2 * mt2 + sub) * P
                    nc.tensor.matmul(
                        ps[:, sub, :],
                        k_fm[:DH, h, m0:m0 + P],
                        q_fm[:DH, h, nsl],
                        start=True, stop=True,
                    )
                nc.scalar.activation(
                    out=e_t[:, 2 * mt2:2 * mt2 + 2, :], in_=ps[:],
                    func=mybir.ActivationFunctionType.Exp,
                    bias=esh_t[:], scale=SCALE / (S_A * S_A),
                )

        def emit_pv(h, nh, e_t):
            po = work1.tile([P, NC_], F32, tag="w", name=f"po_{h}_{nh}")
            for mp in range(NT // 2):
                nc.tensor.matmul(
                    po[:VW + 1, :],
                    v_ext[:, 2 * mp:2 * mp + 2, h, 0:VW + 1],
                    e_t[:, 2 * mp:2 * mp + 2, :],
                    start=(mp == 0), stop=(mp == NT // 2 - 1),
                    perf_mode=DR,
                )
            rs = rrow.tile([1, NC_], F32, tag="rs", name=f"rs_{h}_{nh}")
            nc.vector.reciprocal_approx_fast(out=rs[0:1, :], in_=po[0:1, :])
            rb = rrow.tile([P, NC_], F32, tag="rb", name=f"rb_{h}_{nh}")
            nc.gpsimd.partition_broadcast(rb[:VW + 1, :], rs[0:1, :])
            nc.vector.tensor_mul(
                o_fm[0:VW + 1, h, nh * NC_:(nh + 1) * NC_],
                po[0:VW + 1, :], rb[0:VW + 1, :],
            )

        def emit_proj_c(nt, c0, cw):
            pj = work1.tile([P, NC_], F32, tag="w", name=f"pj_{nt}_{c0}")
            for hp in range(H // 2):
                nc.tensor.matmul(
                    pj[:, :cw],
                    o_fm[0:VW + 1, 2 * hp:2 * hp + 2, nt * P:(nt + 1) * P],
                    wproj[0:VW + 1, 2 * hp:2 * hp + 2, c0:c0 + cw],
                    start=(hp == 0), stop=(hp == H // 2 - 1),
                    perf_mode=DR,
                )
            nc.vector.scalar_tensor_tensor(
                x_tok[:, nt, c0:c0 + cw],
                pj[:, :cw], 1.0 / (S_A * S_W), x_tok[:, nt, c0:c0 + cw],
                MULT, ADD,
            )

        def emit_proj(nt):
            emit_proj_c(nt, 0, 512)
            emit_proj_c(nt, 512, 256)

        def emit_fc1_mms(pg, ff, half):
            """fc1 contraction: kt 0-3 bf16, kt 4-5 fp8 DoubleRow at scale
            1 (same psum scale, so the accumulation group mixes freely)."""
            wfc1, wfc18 = _mlp_weights[0], _mlp_weights[1]
            for kt in range(4):
                nc.tensor.matmul(
                    pg[:],
                    wfc1[:, kt, ff * P:(ff + 1) * P],
                    h2_fm[:, kt, half * NC_:(half + 1) * NC_],
                    start=(kt == 0), stop=False,
                )
            nc.tensor.matmul(
                pg[:],
                wfc18[:, 0:2, ff * P:(ff + 1) * P],
                h2_fm8[:, 0:2, half * NC_:(half + 1) * NC_],
                start=False, stop=True, perf_mode=DR,
            )

        def emit_fc1_park(g_t, half, ff0, ff1):
            """fc1 matmuls early (under the exp-bound attention window),
            parking the pre-activation in g_t as bf16; gelu (ACT) runs
            later from SBUF so the exp table set stays resident."""
            for ff in range(ff0, ff1):
                pg = work1.tile([P, NC_], F32, tag="w", name=f"pk_{half}_{ff}")
                emit_fc1_mms(pg, ff, half)
                nc.vector.tensor_copy(g_t[:, ff, :], pg[:])

        def emit_gelu_parked(g_t, ff0, ff1):
            for ff in range(ff0, ff1):
                nc.scalar.activation(
                    out=g_t[:, ff, :], in_=g_t[:, ff, :],
                    func=mybir.ActivationFunctionType.Gelu,
                    bias=bf1c[:, ff:ff + 1], scale=1.0,
                )

        _mlp_pools = []
        _mlp_weights = []

        def _alloc_mlp_weights():
            wqkvp.release()
            hfmp.release()
            wfc1p = tc.alloc_tile_pool(name="wfc1p", bufs=1, side="left")
            w1 = wfc1p.tile([P, 4, DFF], BF16, name="wfc1")
            w18 = wfc1p.tile([P, 2, DFF], F8, name="wfc18")
            w1r = wfc1_d.rearrange("(kt p) f -> p kt f", p=P)
            w18r = wfc18_d.rearrange("(kt p) f -> p kt f", p=P)
            nc.sync.dma_start(w1[:, 0:2, :], w1r[:, 0:2, :])
            nc.gpsimd.dma_start(w1[:, 2:4, :], w1r[:, 2:4, :])
            nc.sync.dma_start(w18[:], w18r[:])
            _mlp_weights.append(w1)
            _mlp_weights.append(w18)
            return [wfc1p]

        def _alloc_wfc2():
            # deferred past the q/k release so the SBUF high-water during
            # attention can afford a 4-deep e_t ring instead
            wfc2p = tc.alloc_tile_pool(name="wfc2p", bufs=1, side="left")
            w2 = wfc2p.tile([P, 20, C], BF16, name="wfc2")
            w28 = wfc2p.tile([P, 4, C], F8, name="wfc28")
            w2r = wfc2_d.rearrange("(t p) c -> p t c", p=P)
            w28r = wfc28_d.rearrange("(t p) c -> p t c", p=P)
            nc.sync.dma_start(w2[:, 0:10, :], w2r[:, 0:10, :])
            nc.gpsimd.dma_start(w2[:, 10:20, :], w2r[:, 10:20, :])
            nc.sync.dma_start(w28[:], w28r[:])
            _mlp_weights.append(w2)
            _mlp_weights.append(w28)
            return [wfc2p]

        # ---------------- LN1 (batched rstd) + qkv ----------------------
        mvs1 = lnscr.tile([P, NT, 2], F32, tag="mvs")
        rst1 = lnscr.tile([P, NT], F32, tag="rst")
        acc_sq = lnscr.tile([P, 2], F32, tag="acq")
        acc_s = lnscr.tile([P, 2], F32, tag="acs")
        m2t = lnscr.tile([P, 2], F32, tag="m2t")

        def emit_ln_stats_act(nt, j):
            """LN stats on the ACT engine (Square/Identity + accum_out),
            freeing the DVE for the parallel bn_stats of other tiles during
            the startup-critical window."""
            scr = lnscr.tile([P, C], BF16, tag="scr")
            nc.scalar.activation(
                out=scr[:], in_=x_tok[:, nt, :],
                func=mybir.ActivationFunctionType.Square,
                bias=0.0, scale=1.0, accum_out=acc_sq[:, j:j + 1],
            )
            scr2 = lnscr.tile([P, C], BF16, tag="scr")
            nc.scalar.activation(
                out=scr2[:], in_=x_tok[:, nt, :],
                func=mybir.ActivationFunctionType.Identity,
                bias=0.0, scale=1.0, accum_out=acc_s[:, j:j + 1],
            )

        def emit_ln_combine_act(w):
            """mu = sum/C, var = sumsq/C - mu^2 for the ACT-stats tiles."""
            nc.vector.tensor_scalar_mul(
                mvs1[:, 0:w, 0], acc_s[:, 0:w], 1.0 / C
            )
            nc.vector.tensor_mul(
                m2t[:, 0:w], mvs1[:, 0:w, 0], mvs1[:, 0:w, 0]
            )
            nc.vector.scalar_tensor_tensor(
                mvs1[:, 0:w, 1], acc_sq[:, 0:w], 1.0 / C, m2t[:, 0:w],
                MULT, mybir.AluOpType.subtract,
            )

        def emit_ln_apply_dve(nt, j, rstds, dst_fm):
            nmu = lnscr.tile([P, 1], F32, tag="nmu1")
            nc.vector.tensor_scalar_mul(nmu[:], mvs1[:, j, 0:1], -1.0)
            h_t = lnscr.tile([P, C], BF16, tag="h")
            nc.vector.tensor_scalar(
                h_t[:], x_tok[:, nt, :], nmu[:], rstds[:, j:j + 1], ADD, MULT
            )
            for kt in range(KT):
                tp = tpps.tile([P, P], BF16, tag="tp")
                nc.tensor.transpose(
                    tp[:], h_t[:, kt * P:(kt + 1) * P], ident[:]
                )
                nc.vector.tensor_copy(
                    dst_fm[:, kt, nt * P:(nt + 1) * P], tp[:]
                )

        def emit_ln_apply(nt, j, rstds, dst_fm):
            """h = (x - mu) * rstd * 16 on ACT (identity: scale/bias, lives
            in every table set so no swap), then PE-transpose into dst_fm
            (fp8, scale 16).  psum->h_fm copies go to gpsimd (DVE is the
            startup critical path)."""
            nmr = lnscr.tile([P, 1], F32, tag="nmr")
            nc.vector.scalar_tensor_tensor(
                nmr[:], mvs1[:, j, 0:1], -1.0, rstds[:, j:j + 1], MULT, MULT
            )
            h_t = lnscr.tile([P, C], BF16, tag="h")
            nc.scalar.activation(
                out=h_t[:], in_=x_tok[:, nt, :],
                func=mybir.ActivationFunctionType.Identity,
                bias=nmr[:], scale=rstds[:, j:j + 1],
            )
            for kt in range(KT):
                tp = tpps.tile([P, P], BF16, tag="tp")
                nc.tensor.transpose(
                    tp[:], h_t[:, kt * P:(kt + 1) * P], ident[:]
                )
                nc.vector.tensor_copy(
                    dst_fm[:, kt, nt * P:(nt + 1) * P], tp[:]
                )

        with tc.high_priority():
            emit_ln_stats_act(0, 0)
            emit_ln_stats_act(1, 1)
            emit_ln_stats(2, mvs1[:, 2, :])
            emit_ln_stats(3, mvs1[:, 3, :])
            emit_ln_combine_act(2)
            emit_newton(mvs1[:, 0:2, 1], rst1[:, 0:2],
                        2, iters=1, final_scale=S_A)
            emit_ln_apply(0, 0, rst1, h_fm)
            emit_ln_apply(1, 1, rst1, h_fm)
            emit_newton(mvs1[:, 2:4, 1], rst1[:, 2:4],
                        2, iters=1, final_scale=S_A)
            emit_ln_apply_dve(2, 2, rst1, h_fm)
            emit_ln_apply_dve(3, 3, rst1, h_fm)
        # q over the first token half only needs LN1 of tiles 0-3;
        # LN1 of tiles 4-7 interleaves on DVE/ACT under these matmuls.
        for nt in range(4, NT):
            emit_ln_stats(nt, mvs1[:, nt, :])
        emit_newton(mvs1[:, 4:8, 1], rst1[:, 4:8], 4, iters=1,
                    final_scale=S_A)
        emit_ln_apply(4, 4, rst1, h_fm)
        emit_ln_apply(5, 5, rst1, h_fm)
        for h in range(8):
            emit_qk1(h, 0, 0)
            if h == 0:
                emit_ln_apply(6, 6, rst1, h_fm)
            elif h == 1:
                emit_ln_apply(7, 7, rst1, h_fm)
            elif h >= 4:
                emit_qk1(h - 4, 0, 1)
        # k (both halves) with a deep trickle of score fills: e tiles are
        # fp8 now, so an 8-deep e ring lets ACT start the exp stream ~2.5
        # slots early, shrinking the exp-bound attention window.
        slots = [(i, nh) for nh in (0, 1) for i in range(4)]
        e_live = {}

        def slot_begin(s):
            i, nh = slots[s]
            e_live[s] = (emit_scores_begin(2 * i, nh),
                         emit_scores_begin(2 * i + 1, nh))

        def slot_fill(s, mt2):
            i, nh = slots[s]
            eA, eB = e_live[s]
            emit_scores_fill_pair(eA, eB, i, nh, mt2)

        def slot_pv(s):
            i, nh = slots[s]
            eA, eB = e_live.pop(s)
            emit_pv(2 * i, nh, eA)
            emit_pv(2 * i + 1, nh, eB)

        for h in range(8):
            emit_qk1(h, 1, 0)
            emit_qk1(h, 1, 1)
            if h == 2:
                slot_begin(0)
                slot_fill(0, 0)
            elif h == 3:
                slot_fill(0, 1)
            elif h == 4:
                slot_fill(0, 2)
            elif h == 5:
                slot_fill(0, 3)
            elif h == 6:
                slot_begin(1)
                slot_fill(1, 0)
            elif h == 7:
                slot_fill(1, 1)
        for nt in range(NT):
            emit_badd(nt, bpb)
        # all of v must precede the first PV (PV contracts over all of it)
        for nt in range(NT):
            emit_v(nt, 0)
            emit_v(nt, 1)
            if nt == 0:
                slot_fill(1, 2)
            elif nt == 2:
                slot_fill(1, 3)
            elif nt == 3:
                slot_begin(2)
                slot_fill(2, 0)
            elif nt == 4:
                slot_fill(2, 1)
            elif nt == 5:
                slot_fill(2, 2)
            elif nt == 6:
                slot_fill(2, 3)
            elif nt == 7:
                slot_begin(3)
                slot_fill(3, 0)

        # steady state: slot s fills + PV of slot s-2 (ACT runs ~2 slots
        # behind PE by design), with PE filler (remaining q-nh1, proj, LN2,
        # parked fc1) soaking up the exp-bound slack.
        g0 = gpool.tile([P, 20, NC_], BF16, tag="g", name="g_0")
        g08 = gpool.tile([P, 4, NC_], F8, tag="g8", name="g8_0")
        for s in range(2, 8):
            if s == 2:
                emit_qk1(4, 0, 1)
                emit_qk1(5, 0, 1)
            elif s == 3:
                slot_fill(3, 1)
                emit_qk1(6, 0, 1)
                slot_fill(3, 2)
                emit_qk1(7, 0, 1)
                slot_fill(3, 3)
            else:
                slot_begin(s)
                slot_fill(s, 0)
                slot_fill(s, 1)
                slot_fill(s, 2)
                slot_fill(s, 3)
            slot_pv(s - 2)
            if s == 4:
                # wqkv/h_fm fully consumed: free the space, start MLP loads.
                _mlp_pools.extend(_alloc_mlp_weights())
            elif s == 5:
                emit_proj(0)
                emit_proj(1)
            elif s == 6:
                emit_proj(2)
                emit_ln2(0)
                emit_proj(3)
                emit_ln2(1)
            elif s == 7:
                emit_ln2(2)
                emit_ln2(3)
                emit_fc1_park(g0, 0, 0, 4)
        slot_pv(6)
        emit_fc1_park(g0, 0, 4, 8)
        slot_pv(7)

        if "o_fm" in tap_d:
            nc.sync.dma_start(
                tap_d["o_fm"].rearrange("(h p) n -> p h n", p=P), o_fm[:]
            )
        kpool.release()
        qpool.release()
        _mlp_pools.extend(_alloc_wfc2())
        epool.release()

        # ---------------- MLP -------------------------------------------
        wfc1, wfc18, wfc2, wfc28 = _mlp_weights
        g1pool = tc.alloc_tile_pool(name="g1pool", bufs=1, side="right")
        outs = tc.alloc_tile_pool(name="outs", bufs=2, side="right")

        def emit_fc1_chunk(g_t, g8_t, half, ff0, ff1):
            for ff in range(ff0, ff1):
                pg = work1.tile([P, NC_], F32, tag="w", name=f"pg_{half}_{ff}")
                emit_fc1_mms(pg, ff, half)
                dst = g_t[:, ff, :] if ff < 20 else g8_t[:, ff - 20, :]
                nc.scalar.activation(
                    out=dst, in_=pg[:],
                    func=mybir.ActivationFunctionType.Gelu,
                    bias=bf1c[:, ff:ff + 1], scale=1.0,
                )

        def emit_fc2(q, g_t, g8_t):
            qoff = (q % 2) * 256
            pa = [x2a.tile([P, 512], F32, tag="a", name=f"pa{q}_{j}")
                  for j in range(2)]
            pb = [x2b.tile([P, 256], F32, tag="b", name=f"pb{q}_{j}")
                  for j in range(2)]
            for ff in range(20):
                for j in range(2):
                    lhsT = g_t[:, ff, qoff + j * P:qoff + (j + 1) * P]
                    nc.tensor.matmul(
                        pa[j][:], lhsT, wfc2[:, ff, 0:512],
                        start=(ff == 0), stop=False,
                    )
                    nc.tensor.matmul(
                        pb[j][:], lhsT, wfc2[:, ff, 512:768],
                        start=(ff == 0), stop=False,
                    )
            for f2 in range(2):
                for j in range(2):
                    lhsT8 = g8_t[:, 2 * f2:2 * f2 + 2,
                                 qoff + j * P:qoff + (j + 1) * P]
                    nc.tensor.matmul(
                        pa[j][:], lhsT8, wfc28[:, 2 * f2:2 * f2 + 2, 0:512],
                        start=False, stop=(f2 == 1),
                        perf_mode=DR,
                    )
                    nc.tensor.matmul(
                        pb[j][:], lhsT8, wfc28[:, 2 * f2:2 * f2 + 2, 512:768],
                        start=False, stop=(f2 == 1),
                        perf_mode=DR,
                    )
            yq = [nc.sync, nc.gpsimd, nc.scalar, nc.sync]
            for j in range(2):
                nt = 2 * q + j
                o_t = outs.tile([P, C], BF16, tag="y", name=f"y_{q}_{j}")
                nc.vector.tensor_add(
                    o_t[:, 0:512], pa[j][:], x_tok[:, nt, 0:512]
                )
                yq[nt % 4].dma_start(
                    y_d[nt * P:(nt + 1) * P, 0:512], o_t[:, 0:512]
                )
                nc.vector.tensor_add(
                    o_t[:, 512:768], pb[j][:], x_tok[:, nt, 512:768]
                )
                yq[(nt + 1) % 4].dma_start(
                    y_d[nt * P:(nt + 1) * P, 512:768], o_t[:, 512:768]
                )

        emit_gelu_parked(g0, 0, 8)
        emit_fc1_chunk(g0, g08, 0, 8, 12)
        emit_proj(4)
        emit_fc1_chunk(g0, g08, 0, 12, 16)
        emit_proj(5)
        emit_fc1_chunk(g0, g08, 0, 16, 20)
        emit_proj(6)
        emit_fc1_chunk(g0, g08, 0, 20, 24)
        emit_proj(7)
        for nt in range(4, 8):
            emit_ln2(nt)

        if "x1" in tap_d:  # note: includes +b_fc2 (folded early)
            nc.sync.dma_start(
                tap_d["x1"].rearrange("(nt p) c -> p nt c", p=P), x_tok[:]
            )
        if "h2_fm" in tap_d:
            nc.sync.dma_start(
                tap_d["h2_fm"].rearrange("(kt p) n -> p kt n", p=P), h2_fm[:]
            )

        sps.release()
        tpps.release()
        x2a = tc.alloc_tile_pool(name="x2a", bufs=4, space="PSUM")
        x2b = tc.alloc_tile_pool(name="x2b", bufs=2, space="PSUM")
        emit_fc2(0, g0, g08)
        emit_fc2(1, g0, g08)
        g1 = g1pool.tile([P, 20, NC_], BF16, tag="g", name="g_1")
        g18 = g1pool.tile([P, 4, NC_], F8, tag="g8", name="g8_1")
        emit_fc1_chunk(g1, g18, 1, 0, 24)
        emit_fc2(2, g1, g18)
        emit_fc2(3, g1, g18)

        x2b.release()
        x2a.release()
        work1.release()
        outs.release()
        g1pool.release()
        for pool in reversed(_mlp_pools):
            pool.release()
        gpool.release()
        vpool.release()
        rrow.release()
        opool.release()
        lnscr.release()
        h2p.release()
        wprojp.release()
        xpool.release()
        consts.release()

    nc.compile()
    return nc


def _prep_inputs(inputs):
    """Host-side prep (exact refactoring of LN gains/biases into weights,
    fp8 quantization of the attention weights)."""
    f = lambda k: np.asarray(inputs[k], dtype=np.float32)
    x = f("x")
    w_qkv, w_proj, w_fc1, w_fc2 = f("w_qkv"), f("w_proj"), f("w_fc1"), f("w_fc2")
    ln1_g, ln1_b, ln2_g, ln2_b = f("ln1_g"), f("ln1_b"), f("ln2_g"), f("ln2_b")
    b_proj, b_fc1, b_fc2 = f("b_proj"), f("b_fc1"), f("b_fc2")

    bf = ml_dtypes.bfloat16
    f8 = ml_dtypes.float8_e4m3

    def q8(w, scale):
        return np.ascontiguousarray(
            np.clip(w * scale, -240.0, 240.0).astype(f8)
        )

    w_qkv_e = ln1_g[:, None] * w_qkv
    qkv_bias = ln1_b @ w_qkv  # [2304]
    qk_bias = np.zeros((P, 2 * H), dtype=np.float32)
    for which in range(2):
        for h in range(H):
            qk_bias[0:DH, which * H + h] = S_A * qkv_bias[
                which * C + h * DH: which * C + (h + 1) * DH
            ]
    vb = qkv_bias[2 * C: 3 * C]  # v bias passes through softmax additively
    b_proj_e = b_proj + vb @ w_proj
    # head-aligned w_proj rows: block h rows 1..96 (row 0 pairs with colsum row)
    w_proj_p = np.zeros((H * P, C), dtype=np.float32)
    for h in range(H):
        w_proj_p[h * P + 1: h * P + 1 + DH, :] = w_proj[h * DH:(h + 1) * DH, :]
    w_fc1_e = ln2_g[:, None] * w_fc1
    b_fc1_e = b_fc1 + ln2_b @ w_fc1

    common = {
        "w_qkv_e": q8(w_qkv_e, S_W),
        "w_proj_p": q8(w_proj_p, S_W),
        "w_fc1_e": np.ascontiguousarray(w_fc1_e.astype(bf)),
        "w_fc1_8": np.ascontiguousarray(
            np.clip(w_fc1_e[512:768], -240.0, 240.0).astype(f8)
        ),
        "w_fc2": np.ascontiguousarray(w_fc2.astype(bf)),
        "w_fc2_8": np.ascontiguousarray(
            np.clip(w_fc2[2560:3072], -240.0, 240.0).astype(f8)
        ),
        "qk_bias": qk_bias,
        "b_proj_e": np.ascontiguousarray(b_proj_e.astype(bf)),
        "b_fc1_e": b_fc1_e,
        "b_fc2": np.ascontiguousarray(b_fc2.astype(bf)),
    }
    xb = x.astype(bf)
    return [dict(common, x_bf=np.ascontiguousarray(xb[i])) for i in range(8)]


def kernel(**inputs):
    if "nc" not in _CACHED:
        _CACHED["nc"] = build()
    nc = _CACHED["nc"]
    in_maps = _prep_inputs(inputs)
    res = run_bass_kernel_spmd(nc, in_maps, core_ids=list(range(8)))
    out = np.stack([res.results[i]["y"] for i in range(8)], axis=0)
    return out.astype(np.float32)
